# revision 1
# baseline (speedup 1.0000x reference)
"""Group-equivariant conv (folds to dense 128->128ch 3x3 conv, pad=1) on 8 trn2 cores.

Strategy: data-parallel over batch (2 images/core). The G^2-shifted group conv
is expanded on host (pure index shuffle, no FLOPs) into a dense [128,128,3,3]
weight. On device each image is laid out channel-on-partition as a zero-padded
flattened plane [128, 130*130]; the 3x3 conv is 9 PSUM-accumulated matmuls per
output chunk, where tap (dy,dx)'s rhs is just a constant-offset slice of the
flattened plane. Matmuls run bf16 (inputs cast inline by the SWDGE DMA,
weights cast on host; fp32 PSUM accumulation) — ~169ns per N=390 matmul vs
183ns for fp32r. Measured ~150us on HW; PE-bound (matmul stream is 130us,
memory roofline 94us, fixed preamble+epilogue ~12us).
"""

import sys

for _p in ("/opt/trn_rl_repo",):
    if _p not in sys.path:
        sys.path.insert(0, _p)

from contextlib import ExitStack

import numpy as np

import concourse.bacc as bacc
import concourse.mybir as mybir
import concourse.tile as tile
from concourse import bass_utils as _bass_utils
from concourse.bass_utils import run_bass_kernel_spmd

# Walrus's LDWEIGHTS-dedup pass stays off: for fp32r standalone-LDW is a known
# all-zeros hazard, and for bf16 the rewritten standalone InstLdweights fails
# walrus codegen (visitInstLdweights internal error). Measured no win anyway.
_ENABLE_LDW_OPT = False

_orig_run_command = _bass_utils.run_command


def _patched_run_command(argv, **kwargs):
    if _ENABLE_LDW_OPT and isinstance(argv, list):
        argv = [
            "--enable-ldw-opt=true" if a == "--enable-ldw-opt=false" else a
            for a in argv
        ]
    return _orig_run_command(argv, **kwargs)


_bass_utils.run_command = _patched_run_command

NCORES = 8
B, C, H, W = 16, 128, 128, 128
BPC = B // NCORES           # images per core
S = W + 2                   # padded row stride
XCOLS = (H + 2) * S + 4     # padded plane + tail guard for last tap reads
CH = 3                      # output rows per PSUM chunk (N = 3*130 = 390 <= 512)
# input row-block schedule per image: image 0 front-loads small blocks so the
# first matmul's gate (block 0 completion) clears ASAP.
BLOCKS_IMG0 = [4, 4] + [8] * 15
BLOCKS_IMGN = [8] * 16
# chunks-per-PSUM-group schedule: image 0 ramps up so the PE can start as soon
# as the first input rows land (taps-outer consumes a whole group's rows within
# the first tap pass); both images ramp down so the post-last-matmul tail is a
# tiny copy+DMA. 43 chunks per image.
GROUPS_IMG0 = [1, 1, 2, 2, 4, 4, 8, 8, 8, 4, 1]
GROUPS_IMGN = [8, 8, 8, 8, 8, 2, 1]

F32 = mybir.dt.float32
F32R = mybir.dt.float32r
BF16 = mybir.dt.bfloat16

# Moving-operand dtype for the matmuls. fp32r: exact fp32 storage, ~183ns/MM
# measured. bf16: casts inputs on load (SWDGE inline cast), ~1 cyc/col stream
# rate, ~10x larger rounding error (still ~1e-3 rel-to-scale).
MM_BF16 = True


def _expand_weight(weight: np.ndarray) -> np.ndarray:
    """[32,32,4,3,3] -> lhsT layout [ci=128, tap=9, co=128] flattened [128, 1152]."""
    o, i, g, kh, kw = weight.shape
    gi = np.arange(g)
    shift = (gi[:, None] - gi[None, :]) % g            # [g, h]
    wb = weight[:, :, shift]                           # [o, i, g, h, kh, kw]
    wb = np.transpose(wb, (2, 0, 1, 3, 4, 5))          # [g, o, i, h, kh, kw]
    wb = wb.reshape(g * o, i * g, kh, kw)              # [co=128, ci=128, 3, 3]
    wt = np.transpose(wb, (1, 2, 3, 0))                # [ci, kh, kw, co]
    return np.ascontiguousarray(wt.reshape(C, 9 * C)).astype(np.float32)


def _chunks():
    out = []
    y = 0
    while y < H:
        rows = min(CH, H - y)
        out.append((y, rows))
        y += rows
    return out


def _build_body(ctx: ExitStack, tc: tile.TileContext, x_ap, wt_ap, out_ap):
    nc = tc.nc
    mmdt = BF16 if MM_BF16 else F32R
    xpool = ctx.enter_context(tc.tile_pool(name="xp", bufs=1))
    wpool = ctx.enter_context(tc.tile_pool(name="wp", bufs=1))
    opool = ctx.enter_context(tc.tile_pool(name="op", bufs=3))
    ppool = ctx.enter_context(tc.tile_pool(name="pp", bufs=8, space="PSUM"))

    wt = wpool.tile([C, 9 * C], mmdt, name="wt_sb")
    # ACT ring (keeps the SP ring free so input block 0 starts immediately;
    # HWDGE rings are FIFO per issuing engine). Tap 0 goes first on its own so
    # the first matmul's weight gate clears after 64KB, not 590KB.
    nc.scalar.dma_start(out=wt[:, 0:C], in_=wt_ap[:, 0:C])
    nc.scalar.dma_start(out=wt[:, C:9 * C], in_=wt_ap[:, C:9 * C])

    xbufs = []
    for i in range(BPC):
        xb = xpool.tile([C, XCOLS], mmdt, name=f"xb{i}", tag=f"xb{i}")
        xbufs.append(xb)
        # Zero only the pad cells once; interior DMAs never touch them.
        # (memset can't encode float32r — bitcast those APs to plain f32.)
        cast = (lambda ap: ap) if MM_BF16 else (lambda ap: ap.bitcast(F32))
        nc.vector.memset(cast(xb[:, 0:S]), 0.0)                   # top pad row
        nc.vector.memset(cast(xb[:, (H + 1) * S:XCOLS]), 0.0)     # bottom row + guard
        pairs = xb[:, S - 1:S - 1 + (H + 1) * S].rearrange(
            "p (r s) -> p r s", s=S)[:, :, 0:2]                   # col pads (row ends)
        nc.vector.memset(cast(pairs), 0.0)

    chunks = _chunks()

    for img in range(BPC):
        sched = GROUPS_IMG0 if img == 0 else GROUPS_IMGN
        assert sum(sched) == len(chunks)
        groups = []
        i = 0
        for gs in sched:
            groups.append(chunks[i:i + gs])
            i += gs
        xb = xbufs[img]
        xview = xb[:, 0:(H + 2) * S].rearrange("p (r s) -> p r s", s=S)
        r0 = 0
        for bi, rb in enumerate(BLOCKS_IMG0 if img == 0 else BLOCKS_IMGN):
            dst = xview[:, 1 + r0:1 + r0 + rb, 1:1 + W]
            src = x_ap[img, :, r0:r0 + rb, :]
            if not MM_BF16:
                nc.sync.dma_start(out=dst, in_=src)
            else:
                # SWDGE casts f32 -> bf16 inline during the transfer.
                nc.gpsimd.dma_start(out=dst, in_=src)
            r0 += rb

        for grp in groups:
            g_y0 = grp[0][0]
            g_rows = sum(r for _, r in grp)
            psums = [ppool.tile([C, 512], F32, name="ps", tag="ps") for _ in grp]
            for t in range(9):
                dy, dx = divmod(t, 3)
                wslice = wt[:, t * C:(t + 1) * C]
                for pt, (y, rows) in zip(psums, grp):
                    n = rows * S
                    off = (y + dy) * S + dx
                    nc.tensor.matmul(
                        pt[:, 0:n], wslice, xb[:, off:off + n],
                        start=(t == 0), stop=(t == 8),
                    )
            stage = opool.tile([C, g_rows * W], F32, name="stage", tag="stage")
            col = 0
            for pt, (y, rows) in zip(psums, grp):
                src = pt[:, 0:rows * S].rearrange("p (r s) -> p r s", s=S)[:, :, 0:W]
                dst = stage[:, col:col + rows * W].rearrange("p (r s) -> p r s", s=W)
                nc.vector.tensor_copy(dst, src)
                col += rows * W
            # Stores go on the ACT HWDGE ring so they never queue behind the
            # (large) input loads on the SP ring.
            nc.scalar.dma_start(
                out=out_ap[img, :, g_y0:g_y0 + g_rows, :],
                in_=stage[:, 0:g_rows * W],
            )


_NC_CACHE = None


def _get_nc():
    global _NC_CACHE
    if _NC_CACHE is None:
        nc = bacc.Bacc("TRN2", target_bir_lowering=False, debug=False)
        xdt = F32 if MM_BF16 else F32R
        wdt = BF16 if MM_BF16 else F32R
        x_ap = nc.dram_tensor("x", [BPC, C, H, W], xdt, kind="ExternalInput").ap()
        wt_ap = nc.dram_tensor("wt", [C, 9 * C], wdt, kind="ExternalInput").ap()
        out_ap = nc.dram_tensor("out", [BPC, C, H, W], F32, kind="ExternalOutput").ap()
        with tile.TileContext(nc) as tc:
            with ExitStack() as ctx:
                _build_body(ctx, tc, x_ap, wt_ap, out_ap)
        nc.compile()
        _NC_CACHE = nc
    return _NC_CACHE


def _run(x: np.ndarray, weight: np.ndarray, trace: bool = False, **kw):
    x = np.ascontiguousarray(np.asarray(x, dtype=np.float32))
    wt = _expand_weight(np.asarray(weight, dtype=np.float32))
    if MM_BF16:
        import ml_dtypes
        wt = wt.astype(ml_dtypes.bfloat16)
    nc = _get_nc()
    in_maps = [
        {"x": x[c * BPC:(c + 1) * BPC], "wt": wt} for c in range(NCORES)
    ]
    res = run_bass_kernel_spmd(nc, in_maps, list(range(NCORES)), trace=trace, **kw)
    out = np.concatenate([res.results[c]["out"] for c in range(NCORES)], axis=0)
    return out, res


def kernel(x: np.ndarray, weight: np.ndarray) -> np.ndarray:
    out, _ = _run(x, weight)
    return out



# revision 7
# speedup vs baseline: 1.0354x; 1.0354x over previous
"""Group-equivariant conv (dense 128->128ch 3x3, pad=1) on 8 trn2 cores.

Data-parallel over batch (2 images/core). Hybrid algorithm per image:
  - rows 0..D-1: direct conv — 9 PSUM-accumulated matmuls per 3-row chunk
    (baseline scheme). These run first and warm the PE while x streams in.
  - remaining (128-D)/2 row-pairs: F(2,3) Winograd along y — DVE computes
    4 transformed input planes V_k (bf16 tensor_tensor, 2x perf mode), PE
    does 12 matmuls per 3-pair chunk instead of 18 (contraction ci=128,
    PSUM-accumulated over the 3 x-taps), DVE combines the 4 fp32 PSUM
    m-planes into 2 output rows per pair (A^T), ACT does the direct-path
    PSUM->SBUF copies and issues store DMAs.
x is cast to bf16 on host (halves input HBM traffic vs the fp32+SWDGE-cast
baseline); weights are expanded + G-transformed + bf16-cast on host.
Output stays fp32 end-to-end (PSUM -> DVE -> HBM).
"""

import sys

for _p in ("/opt/trn_rl_repo",):
    if _p not in sys.path:
        sys.path.insert(0, _p)

from contextlib import ExitStack

import numpy as np

import concourse.bacc as bacc
import concourse.mybir as mybir
import concourse.tile as tile
from concourse.bass_utils import run_bass_kernel_spmd

NCORES = 8
B, C, H, W = 16, 128, 128, 128
BPC = B // NCORES           # images per core
S = W + 2                   # padded row stride
NROWS = H + 2               # padded plane rows
XCOLS = NROWS * S + 4       # padded plane + tail guard for tap reads
D = 24                      # direct-conv rows at the top of each image (mult of 6)
WP = (H - D) // 2           # winograd row-pairs per image
CH = 3                      # rows (direct) / pairs (wino) per PSUM chunk
BAND = 15                   # pairs per V band (multiple of CH)
ZCOL = BAND * S + 4         # V-plane stride within a band tile
# input row-block schedule: front-load small blocks so the first direct
# chunk's gate clears ASAP.
BLOCKS_IMG0 = [4, 4] + [8] * 15
BLOCKS_IMGN = [8] * 16

F32 = mybir.dt.float32
BF16 = mybir.dt.bfloat16

NW_DIR = 9 * C              # direct tap weights: cols [0, 9C)
NW_WINO = 12 * C            # wino weights: cols [9C, 21C)
NWCOLS = NW_DIR + NW_WINO


def _expand_weight(weight: np.ndarray) -> np.ndarray:
    """[32,32,4,3,3] -> [co=128, ci=128, kh, kw] dense equivalent."""
    o, i, g, kh, kw = weight.shape
    gi = np.arange(g)
    shift = (gi[:, None] - gi[None, :]) % g            # [g, h]
    wb = weight[:, :, shift]                           # [o, i, g, h, kh, kw]
    wb = np.transpose(wb, (2, 0, 1, 3, 4, 5))          # [g, o, i, h, kh, kw]
    return wb.reshape(g * o, i * g, kh, kw)            # [co, ci, 3, 3]


def _device_weights(weight: np.ndarray) -> np.ndarray:
    """Build the [128, 21*128] fp32 weight image: direct taps then wino."""
    wb = _expand_weight(weight.astype(np.float32))     # [co, ci, kh, kw]
    # direct: [ci, kh, kw, co] -> [128, 9*128]
    wt_dir = np.transpose(wb, (1, 2, 3, 0)).reshape(C, NW_DIR)
    # wino along y (kh): G-combos, V2 sign absorbed (V2' = d1 - d2)
    w0, w1, w2 = wb[:, :, 0, :], wb[:, :, 1, :], wb[:, :, 2, :]  # [co, ci, kw]
    g = np.stack(
        [w0, (w0 + w1 + w2) * 0.5, (w1 - w0 - w2) * 0.5, w2], axis=0
    )                                                  # [k, co, ci, kw]
    wt_wino = np.transpose(g, (2, 0, 3, 1)).reshape(C, NW_WINO)  # [ci,(k,kw,co)]
    return np.ascontiguousarray(np.concatenate([wt_dir, wt_wino], axis=1))


def _build_image(ctx, tc, pools, x_ap, wt, out_ap, img):
    nc = tc.nc
    xpool, vpool, ppool, tpool, wstpool, dstpool = pools

    xb = xpool.tile([C, XCOLS], BF16, name=f"xb{img}", tag=f"xb{img}")
    xview = xb[:, 0:NROWS * S].rearrange("p (r s) -> p r s", s=S)
    # zero the pad cells once; interior DMAs never touch them
    nc.vector.memset(xb[:, 0:S], 0.0)                     # top pad row
    nc.vector.memset(xb[:, (H + 1) * S:XCOLS], 0.0)       # bottom row + guard
    pairs = xb[:, S - 1:S - 1 + (H + 1) * S].rearrange(
        "p (r s) -> p r s", s=S)[:, :, 0:2]               # col pads (row ends)
    nc.vector.memset(pairs, 0.0)

    # input loads (bf16, plain HWDGE on the sync ring)
    r0 = 0
    for rb in (BLOCKS_IMG0 if img == 0 else BLOCKS_IMGN):
        nc.sync.dma_start(
            out=xview[:, 1 + r0:1 + r0 + rb, 1:1 + W],
            in_=x_ap[img, :, r0:r0 + rb, :],
        )
        r0 += rb

    # --- V bands (DVE, bf16 2x mode) ---
    # plane rows D..129 (=106+ rows) grouped in pairs two ways:
    #   A[q, t] = plane row D + 2q + t        (d0 = A[q,0], d1 = A[q,1])
    #   Bv[q, t] = plane row D + 2 + 2q + t   (d2 = Bv[q,0], d3 = Bv[q,1])
    na = (NROWS - D) // 2
    A = xb[:, D * S:(D + 2 * na) * S].rearrange(
        "p (q t s) -> p q t s", t=2, s=S)
    Bv = xb[:, (D + 2) * S:(D + 2 + 2 * (na - 1)) * S].rearrange(
        "p (q t s) -> p q t s", t=2, s=S)

    vbands = []
    q0 = 0
    while q0 < WP:
        nq = min(BAND, WP - q0)
        vt = vpool.tile([C, 4 * ZCOL], BF16, name="vb", tag="vb")
        # zero the 4-col tail guard of each plane: the last chunk's dx-shifted
        # reads wrap past the last pair's row into it (wrapped values land in
        # the last pair's col-127 output, so they must be the zero right-pad).
        nc.vector.memset(
            vt.rearrange("p (k z) -> p k z", z=ZCOL)[:, :, nq * S:nq * S + 4],
            0.0)
        d0 = A[:, q0:q0 + nq, 0, :]
        d1 = A[:, q0:q0 + nq, 1, :]
        d2 = Bv[:, q0:q0 + nq, 0, :]
        d3 = Bv[:, q0:q0 + nq, 1, :]
        def vdst(k):
            return vt[:, k * ZCOL:k * ZCOL + nq * S].rearrange(
                "p (q s) -> p q s", s=S)
        nc.vector.tensor_sub(vdst(0), d0, d2)             # V0 = d0 - d2
        nc.vector.tensor_add(vdst(1), d1, d2)             # V1 = d1 + d2
        nc.vector.tensor_sub(vdst(2), d1, d2)             # V2' = d1 - d2
        nc.vector.tensor_sub(vdst(3), d1, d3)             # V3 = d1 - d3
        vbands.append((q0, nq, vt))
        q0 += nq

    # --- direct chunks (rows 0..D-1) ---
    for y in range(0, D, CH):
        n = CH * S
        ps = ppool.tile([C, 512], F32, name="ps", tag="ps")
        for t in range(9):
            dy, dx = divmod(t, 3)
            off = (y + dy) * S + dx
            nc.tensor.matmul(
                ps[:, 0:n], wt[:, t * C:(t + 1) * C], xb[:, off:off + n],
                start=(t == 0), stop=(t == 8),
            )
        st = dstpool.tile([C, CH * W], F32, name="dst", tag="dst")
        src = ps[:, 0:n].rearrange("p (r s) -> p r s", s=S)[:, :, 0:W]
        nc.scalar.copy(st.rearrange("p (r w) -> p r w", w=W), src)
        nc.scalar.dma_start(out=out_ap[img, :, y:y + CH, :], in_=st)

    # --- winograd chunks ---
    for (q0, nq, vt) in vbands:
        for qc in range(0, nq, CH):
            nc_pairs = min(CH, nq - qc)
            n = nc_pairs * S
            ms = [ppool.tile([C, 512], F32, name="ps", tag="ps")
                  for _ in range(4)]
            for k in range(4):
                for dx in range(3):
                    wslice = wt[:, NW_DIR + (k * 3 + dx) * C:
                                NW_DIR + (k * 3 + dx + 1) * C]
                    nc.tensor.matmul(
                        ms[k][:, 0:n], wslice,
                        vt[:, k * ZCOL + qc * S + dx:
                           k * ZCOL + qc * S + dx + n],
                        start=(dx == 0), stop=(dx == 2),
                    )
            mv = [m[:, 0:n].rearrange("p (r s) -> p r s", s=S)[:, :, 0:W]
                  for m in ms]
            # DVE tensor_tensor allows at most one PSUM operand, so ACT
            # first stages m1 (ready after the k=1 matmul chain, i.e. early
            # in the chunk); every DVE op then reads one PSUM + one SBUF.
            tt = tpool.tile([C, CH * W], F32, name="t", tag="t")
            aa = tpool.tile([C, CH * W], F32, name="a", tag="a")
            bb = tpool.tile([C, CH * W], F32, name="b", tag="b")
            tv = tt[:, 0:nc_pairs * W].rearrange("p (r w) -> p r w", w=W)
            av = aa[:, 0:nc_pairs * W].rearrange("p (r w) -> p r w", w=W)
            bv = bb[:, 0:nc_pairs * W].rearrange("p (r w) -> p r w", w=W)
            st = wstpool.tile([C, 2 * CH * W], F32, name="wst", tag="wst")
            stv = st[:, 0:2 * nc_pairs * W].rearrange(
                "p (r t w) -> p r t w", t=2, w=W)
            nc.scalar.copy(tv, mv[1])                          # t = m1
            nc.vector.tensor_add(av, tv, mv[0])                # a = t + m0
            nc.vector.tensor_add(stv[:, :, 0, :], av, mv[2])   # y0 = a + m2
            nc.vector.tensor_sub(bv, tv, mv[2])                # b = t - m2
            nc.vector.tensor_sub(stv[:, :, 1, :], bv, mv[3])   # y1 = b - m3
            y0 = D + 2 * (q0 + qc)
            nc.scalar.dma_start(
                out=out_ap[img, :, y0:y0 + 2 * nc_pairs, :],
                in_=st[:, 0:2 * nc_pairs * W],
            )


def _build_body(ctx: ExitStack, tc: tile.TileContext, x_ap, wt_ap, out_ap):
    nc = tc.nc
    xpool = ctx.enter_context(tc.tile_pool(name="xp", bufs=1))
    vpool = ctx.enter_context(tc.tile_pool(name="vp", bufs=4))
    wpool = ctx.enter_context(tc.tile_pool(name="wp", bufs=1))
    tpool = ctx.enter_context(tc.tile_pool(name="tp", bufs=2))
    wstpool = ctx.enter_context(tc.tile_pool(name="wsp", bufs=3))
    dstpool = ctx.enter_context(tc.tile_pool(name="dsp", bufs=3))
    ppool = ctx.enter_context(tc.tile_pool(name="pp", bufs=8, space="PSUM"))

    wt = wpool.tile([C, NWCOLS], BF16, name="wt_sb")
    # first direct tap goes first on its own so the first matmul's weight
    # gate clears after 64KB; ACT ring keeps the SP ring free for x loads.
    nc.scalar.dma_start(out=wt[:, 0:C], in_=wt_ap[:, 0:C])
    nc.scalar.dma_start(out=wt[:, C:NWCOLS], in_=wt_ap[:, C:NWCOLS])

    pools = (xpool, vpool, ppool, tpool, wstpool, dstpool)
    for img in range(BPC):
        _build_image(ctx, tc, pools, x_ap, wt, out_ap, img)


_NC_CACHE = None


def _get_nc():
    global _NC_CACHE
    if _NC_CACHE is None:
        nc = bacc.Bacc("TRN2", target_bir_lowering=False, debug=False)
        x_ap = nc.dram_tensor("x", [BPC, C, H, W], BF16, kind="ExternalInput").ap()
        wt_ap = nc.dram_tensor("wt", [C, NWCOLS], BF16, kind="ExternalInput").ap()
        out_ap = nc.dram_tensor("out", [BPC, C, H, W], F32, kind="ExternalOutput").ap()
        with tile.TileContext(nc) as tc:
            with ExitStack() as ctx:
                _build_body(ctx, tc, x_ap, wt_ap, out_ap)
        nc.compile()
        _NC_CACHE = nc
    return _NC_CACHE


def _run(x: np.ndarray, weight: np.ndarray, trace: bool = False, **kw):
    import ml_dtypes
    xb = np.ascontiguousarray(
        np.asarray(x, dtype=np.float32)).astype(ml_dtypes.bfloat16)
    wtb = _device_weights(np.asarray(weight, dtype=np.float32)).astype(
        ml_dtypes.bfloat16)
    nc = _get_nc()
    in_maps = [
        {"x": xb[c * BPC:(c + 1) * BPC], "wt": wtb} for c in range(NCORES)
    ]
    res = run_bass_kernel_spmd(nc, in_maps, list(range(NCORES)), trace=trace, **kw)
    out = np.concatenate([res.results[c]["out"] for c in range(NCORES)], axis=0)
    return out, res


def kernel(x: np.ndarray, weight: np.ndarray) -> np.ndarray:
    out, _ = _run(x, weight)
    return out
